# revision 10
# baseline (speedup 1.0000x reference)
"""Trainium2 Bass kernel for a relative-position (Music-Transformer style)
attention head.

Full-input contract: kernel(**inputs) takes the complete batch
  inputs_for_keys/values/queries: [8, 2048, 512] f32
  WK/WV/WQ: [512, 64] f32,  E: [2048, 64] f32
and returns Z: [8, 2048, 64] f32.

Sharding: batch (8) across the 8 NeuronCores, pure data parallel.

Per-core algorithm (natural q-on-partitions coordinates):
  Q^T,K^T = (W^T X^T) via PE-transposed X chunks; Q pre-scaled by 1/sqrt(L).
  V natural [k,64] plus a ones column (softmax denominator comes out of the
  AV matmul as output row 64).
  Rel[q,c] = Q[q]. E[c] computed bandwise, cast bf16, bounced through a DRAM
  scratch whose skewed read AP (row stride W-1) realizes the _skew() op.
  Scratch pad columns hold -90 so the strictly-upper (causal-masked) cells of
  diagonal tiles come back as exp(-90) ~= 0.
  Scores psum = QK matmul + identity-matmul injection of the rel tile.
  exp on ACT (bf16 out), PE transpose to A^T, AV accumulation into Z^T
  [65, 512] psum, then final transpose + per-row 1/denominator scale.

Matmul operands use float32r (TF32-class fp32 matmul at full PE rate); the
BIR verifier requires every producer of an f32r matmul operand to emit f32r,
so those SBUF tensors are allocated as f32r and written with f32r out-APs.
"""

import math

import numpy as np

L = 2048
D = 512
ED = 64
P = 128
B = 8
NT = L // P  # 16 row tiles
NDC = D // P  # 4 contraction chunks for the projections
MASK_NEG = -90.0

_CACHE = {}


def _build_bass(l_seq=L):
    import concourse.bass as bass
    import concourse.mybir as mybir
    import concourse.tile as tile
    from concourse import bacc
    from concourse.masks import make_identity

    f32 = mybir.dt.float32
    f32r = mybir.dt.float32r
    bf16 = mybir.dt.bfloat16
    AF = mybir.ActivationFunctionType

    nt = l_seq // P  # row tiles
    nqc = max(1, nt // 4)  # 512-wide q column blocks for the AV stage

    nc = bacc.Bacc()

    xq = nc.dram_tensor("xq", [l_seq, D], f32, kind="ExternalInput")
    xk = nc.dram_tensor("xk", [l_seq, D], f32, kind="ExternalInput")
    xv = nc.dram_tensor("xv", [l_seq, D], f32, kind="ExternalInput")
    wq = nc.dram_tensor("wq", [D, ED], f32, kind="ExternalInput")
    wk = nc.dram_tensor("wk", [D, ED], f32, kind="ExternalInput")
    wv = nc.dram_tensor("wv", [D, ED], f32, kind="ExternalInput")
    e_in = nc.dram_tensor("e", [l_seq, ED], f32, kind="ExternalInput")
    z_out = nc.dram_tensor("z", [l_seq, ED], f32, kind="ExternalOutput")

    # Per-band scratch for the skew bounce. Band i covers q rows
    # [128i, 128i+128); its causal rel columns c_global in [L-128(i+1), L)
    # are stored at c_local in [0, 128(i+1)), plus 128 pad columns of -90.
    scratch = [
        nc.dram_tensor(f"relscratch{i}", [P, (i + 2) * P], bf16) for i in range(nt)
    ]

    with tile.TileContext(nc) as tc:
        with (
            tc.tile_pool(name="singles", bufs=1) as singles,
            tc.tile_pool(name="xtiles", bufs=3) as xtiles,
            tc.tile_pool(name="xt", bufs=1) as xtp,
            tc.tile_pool(name="relst", bufs=2) as relst,
            tc.tile_pool(name="relin", bufs=3) as relin,
            tc.tile_pool(name="anat", bufs=3) as anat,
            tc.tile_pool(name="atp", bufs=2) as atp,
            tc.tile_pool(name="small", bufs=4) as small,
            tc.tile_pool(name="ps", bufs=4, space="PSUM") as psp,
            tc.tile_pool(name="psc", bufs=2, space="PSUM") as pscp,
            tc.tile_pool(name="psz", bufs=2, space="PSUM") as pszp,
        ):
            # ---------------- setup ----------------
            ident_f = singles.tile([P, P], f32)
            make_identity(nc, ident_f[:, :])
            ident = singles.tile([P, P], f32r)
            nc.vector.tensor_copy(ident[:, :], ident_f[:, :])
            ident_b = singles.tile([P, P], bf16)
            nc.vector.tensor_copy(ident_b[:, :], ident_f[:, :])

            negpad = singles.tile([P, P], bf16)
            nc.vector.memset(negpad[:, :], MASK_NEG)

            w_sb = {}
            for name, w in (("q", wq), ("k", wk), ("v", wv)):
                wt = singles.tile([P, NDC, ED], f32r, name=f"w_{name}")
                nc.sync.dma_start(
                    wt[:, :, :],
                    w[:, :].rearrange("(c p) e -> p c e", p=P).bitcast(f32r),
                )
                w_sb[name] = wt
            # fold the 1/sqrt(L) score scale into WQ so both QK and QE
            # logits come out pre-scaled
            nc.vector.tensor_scalar_mul(
                w_sb["q"][:, :, :],
                w_sb["q"][:, :, :].bitcast(f32),
                1.0 / math.sqrt(l_seq),
            )

            # E^T [64, l_seq]
            et_sb = singles.tile([ED, l_seq], f32r)
            for g in range(0, nt, 4):
                gn = min(4, nt - g)
                et_ps = psp.tile([ED, 4 * P], f32r, tag="ps", name="et_ps")
                for b in range(gn):
                    i = g + b
                    etile = xtiles.tile([P, ED], f32r, tag="etile", name="etile")
                    nc.sync.dma_start(
                        etile[:, :], e_in[i * P : (i + 1) * P, :].bitcast(f32r)
                    )
                    nc.tensor.transpose(
                        et_ps[:, b * P : (b + 1) * P],
                        etile[:, :],
                        ident[:, :],
                    )
                nc.vector.tensor_copy(
                    et_sb[:, g * P : (g + gn) * P],
                    et_ps[:, : gn * P].bitcast(f32),
                )

            # ---------------- projections ----------------
            proj_sb = {}
            copy_flip = [0]

            def psum_copy(dst, src):
                copy_flip[0] ^= 1
                if copy_flip[0]:
                    nc.vector.tensor_copy(dst, src)
                else:
                    nc.scalar.copy(dst, src)

            for name, x in (("q", xq), ("k", xk), ("v", xv)):
                xt = xtp.tile([P, NDC, l_seq], f32r, tag="xt", name=f"xt_{name}")
                for i in range(nt):
                    xtile = xtiles.tile([P, D], f32r, tag="xtile", name="xtile")
                    nc.sync.dma_start(
                        xtile[:, :], x[i * P : (i + 1) * P, :].bitcast(f32r)
                    )
                    st = psp.tile([P, D], f32r, tag="ps", name="xt_st")
                    for c in range(NDC):
                        nc.tensor.transpose(
                            st[:, c * P : (c + 1) * P],
                            xtile[:, c * P : (c + 1) * P],
                            ident[:, :],
                        )
                    psum_copy(
                        xt[:, :, i * P : (i + 1) * P],
                        st[:, :].bitcast(f32).rearrange("p (c x) -> p c x", x=P),
                    )
                pt = singles.tile([ED, l_seq], f32r, name=f"pt_{name}")
                for s in range(l_seq // 512):
                    pp = psp.tile([ED, 512], f32, tag="ps", name="proj_ps")
                    for c in range(NDC):
                        nc.tensor.matmul(
                            pp[:, :],
                            lhsT=w_sb[name][:, c, :],
                            rhs=xt[:, c, s * 512 : (s + 1) * 512],
                            start=(c == 0),
                            stop=(c == NDC - 1),
                        )
                    psum_copy(pt[:, s * 512 : (s + 1) * 512], pp[:, :])
                proj_sb[name] = pt

            qt_sb, kt_sb, vt_t = proj_sb["q"], proj_sb["k"], proj_sb["v"]

            # V natural [128, nt, 65]; column 64 = ones for the denominator
            v_sb = singles.tile([P, nt, ED + 1], f32r)
            ones_f = singles.tile([P, 1], f32)
            nc.vector.memset(ones_f[:, :], 1.0)
            nc.vector.tensor_copy(
                v_sb[:, :, ED : ED + 1],
                ones_f[:, 0:1].broadcast_to((P, nt, 1)),
            )
            for g in range(0, nt, 8):
                gn = min(8, nt - g)
                vst = psp.tile([P, 8 * ED], f32r, tag="ps", name="v_st")
                for b in range(gn):
                    j = g + b
                    nc.tensor.transpose(
                        vst[:, b * ED : (b + 1) * ED],
                        vt_t[:, j * P : (j + 1) * P],
                        ident[:ED, :ED],
                    )
                psum_copy(
                    v_sb[:, g : g + gn, 0:ED],
                    vst[:, : gn * ED].bitcast(f32).rearrange(
                        "p (j x) -> p j x", x=ED
                    ),
                )

            # ---------------- rel logits bounce (skew via DRAM) ----------------
            rel_writes = []
            for i in range(nt):
                cw = (i + 1) * P  # causal columns for this band
                c0 = l_seq - cw  # global column of c_local 0
                rst = relst.tile([P, l_seq], bf16, tag="relst", name="rel_stage")
                for s in range(0, cw, 512):
                    w = min(512, cw - s)
                    rp = psp.tile([P, 512], f32, tag="ps", name="rel_ps")
                    nc.tensor.matmul(
                        rp[:, :w],
                        lhsT=qt_sb[:, i * P : (i + 1) * P],
                        rhs=et_sb[:, c0 + s : c0 + s + w],
                        start=True,
                        stop=True,
                    )
                    psum_copy(rst[:, s : s + w], rp[:, :w])
                w1 = nc.sync.dma_start(scratch[i][:, 0:cw], rst[:, 0:cw])
                w2 = nc.sync.dma_start(scratch[i][:, cw : cw + P], negpad[:, :])
                rel_writes.append((w1, w2))

            # ---------------- scores / softmax / AV ----------------
            from concourse.tile_rust import add_dep_helper

            zt_sb = singles.tile([P, l_seq], f32r)

            for qc in range(nqc):
                at_sb = atp.tile([P, nt, 512], f32r, tag="at", name="at_sb")
                for il in range(4):
                    i = qc * 4 + il
                    cw = (i + 1) * P
                    wi = (i + 2) * P  # scratch row length
                    # skewed read: row p starts at local column 127 - p
                    rin = relin.tile([P, l_seq], bf16, tag="relin", name="rel_in")
                    rd = nc.sync.dma_start(
                        rin[:, 0:cw],
                        bass.AP(scratch[i], P - 1, [[wi - 1, P], [1, cw]]),
                    )
                    for wr in rel_writes[i]:
                        add_dep_helper(
                            rd.ins, wr.ins, reason="rel skew read after write"
                        )
                    for s in range(0, cw, 512):
                        w = min(512, cw - s)
                        nb = w // P
                        sc = pscp.tile([P, 512], f32, tag="sc", name="sc_ps")
                        nc.tensor.matmul(
                            sc[:, :w],
                            lhsT=qt_sb[:, i * P : (i + 1) * P],
                            rhs=kt_sb[:, s : s + w],
                            start=True,
                            stop=False,
                        )
                        nc.tensor.matmul(
                            sc[:, :w],
                            lhsT=ident_b[:, :],
                            rhs=rin[:, s : s + w],
                            start=False,
                            stop=True,
                        )
                        an = anat.tile([P, 512], bf16, tag="an", name="a_nat")
                        nc.scalar.activation(an[:, :w], sc[:, :w], AF.Exp)
                        ast = psp.tile([P, 512], bf16, tag="ps", name="at_st")
                        for b in range(nb):
                            nc.tensor.transpose(
                                ast[:, b * P : (b + 1) * P],
                                an[:, b * P : (b + 1) * P],
                                ident_b[:, :],
                            )
                        j0 = s // P
                        psum_copy(
                            at_sb[:, j0 : j0 + nb, il * P : (il + 1) * P],
                            ast[:, :w].rearrange("p (b x) -> p b x", x=P),
                        )
                # AV for this 512-wide q block
                jmax = qc * 4 + 4
                zp = pszp.tile([ED + 1, 512], f32, tag="z", name="z_ps")
                for j in range(jmax):
                    sl = max(0, j - qc * 4) * P
                    nc.tensor.matmul(
                        zp[:, sl:512],
                        lhsT=v_sb[:, j, :],
                        rhs=at_sb[:, j, sl:512],
                        start=(j == 0),
                        stop=(j == jmax - 1),
                    )
                nc.vector.tensor_copy(
                    zt_sb[0 : ED + 1, qc * 512 : (qc + 1) * 512], zp[:, :]
                )

            # ---------------- finalize: transpose + 1/denominator ----------------
            zout_sb = singles.tile([P, nt, ED], f32)
            for i in range(nt):
                zf = psp.tile([P, P], f32r, tag="ps", name="zf_ps")
                nc.tensor.transpose(
                    zf[:, :],
                    zt_sb[:, i * P : (i + 1) * P],
                    ident[:, :],
                )
                rden = small.tile([P, 1], f32, tag="rden", name="rden")
                nc.vector.reciprocal(rden[:, :], zf[:, ED : ED + 1].bitcast(f32))
                nc.vector.tensor_scalar_mul(
                    zout_sb[:, i, :], zf[:, 0:ED].bitcast(f32), rden[:, :]
                )
            nc.sync.dma_start(
                z_out[:, :].rearrange("(i p) e -> p i e", p=P), zout_sb[:, :, :]
            )

    nc.compile()
    return nc


def kernel(**inputs):
    from concourse.bass_utils import run_bass_kernel_spmd

    key = "full"
    if key not in _CACHE:
        _CACHE[key] = _build_bass(L)
    nc = _CACHE[key]

    xq = np.ascontiguousarray(np.asarray(inputs["inputs_for_queries"], np.float32))
    xk = np.ascontiguousarray(np.asarray(inputs["inputs_for_keys"], np.float32))
    xv = np.ascontiguousarray(np.asarray(inputs["inputs_for_values"], np.float32))
    wq = np.ascontiguousarray(np.asarray(inputs["WQ"], np.float32))
    wk = np.ascontiguousarray(np.asarray(inputs["WK"], np.float32))
    wv = np.ascontiguousarray(np.asarray(inputs["WV"], np.float32))
    e = np.ascontiguousarray(np.asarray(inputs["E"], np.float32))

    in_maps = [
        {
            "xq": np.ascontiguousarray(xq[c]),
            "xk": np.ascontiguousarray(xk[c]),
            "xv": np.ascontiguousarray(xv[c]),
            "wq": wq,
            "wk": wk,
            "wv": wv,
            "e": e,
        }
        for c in range(B)
    ]
    res = run_bass_kernel_spmd(nc, in_maps, core_ids=list(range(B)))
    return np.stack([r["z"] for r in res.results], axis=0)
